# revision 2
# baseline (speedup 1.0000x reference)
"""Multi-head attention on 8 TRN2 NeuronCores — head-sharded, v3.

Problem: B=4, Sf=St=2048, DIM=768, H=12, Dh=64, f32 reference.

Sharding: core c = (batch b=c//2, head-group g=c%2); per-core 6 heads x all
2048 queries; host adds the two per-batch partials.

v3 changes vs v2 (the 368us baseline on current clocks):
  - ACT runs exps (the 1.2GHz-floor work) plus only the 6 ctx block drains;
    qt/kt round drains, o-round drains (in-loop), the 1/Z reciprocal and
    bf16 cast all move to the DVE, which has ~20% slack.
  - PE matmul emission is ordered so tile_position-packed groups sit
    adjacent in the PE queue (S pairs row-packed, ctx pairs col-packed,
    Z quads col-packed) — measured to execute concurrently (pair ~= 216ns).
  - v(0..7) projection rounds run in the prologue under the xf DMA wait;
    the in-loop schedule then stays at <= 1 round per step.
  - 10 of the 12 qh=0 output-projection rounds run inside block 5; the
    tail is the remaining 14 rounds with ACT drains (ACT is idle there).
"""

import numpy as np
import ml_dtypes

BF16 = ml_dtypes.bfloat16

B, SF, ST, DIM = 4, 2048, 2048, 768
NH, HD = 12, 64
SCALE = HD ** -0.5
NCORES = 8
NHC = NH // 2           # 6 heads per core
HP = NHC // 2           # 3 head-pairs per core
VW = NHC * HD           # 384: per-core inner width
QROWS = SF              # all 2048 query rows per core
NST = ST // 128         # 16 key tiles
NK = DIM // 128         # 6 contraction chunks

_CACHED_NC = None


def _build_nc():
    from concourse import bacc, tile, mybir
    import concourse.bass as bass

    dt = mybir.dt
    nc = bacc.Bacc("TRN2", target_bir_lowering=False, debug=False,
                   num_devices=NCORES)

    xfT = nc.dram_tensor("xfT", [DIM, QROWS], dt.bfloat16, kind="ExternalInput").ap()
    xtT = nc.dram_tensor("xtT", [DIM, ST], dt.bfloat16, kind="ExternalInput").ap()
    maskT = nc.dram_tensor("maskT", [ST, QROWS], dt.bfloat16, kind="ExternalInput").ap()
    wqkv = nc.dram_tensor("wqkv", [DIM, 3 * VW], dt.bfloat16, kind="ExternalInput").ap()
    wo = nc.dram_tensor("wo", [VW, DIM], dt.bfloat16, kind="ExternalInput").ap()
    biases = nc.dram_tensor("biases", [128, HP + NK], dt.float32, kind="ExternalInput").ap()
    out = nc.dram_tensor("out", [DIM, QROWS], dt.bfloat16, kind="ExternalOutput").ap()

    EXP = mybir.ActivationFunctionType.Exp

    with tile.TileContext(nc) as tc:
        persist_cm = tc.tile_pool(name="persist", bufs=1)
        persist = persist_cm.__enter__()

        qt_sb = [persist.tile([128, QROWS], dt.bfloat16, tag=f"qt{i}", name=f"qt{i}")
                 for i in range(HP)]
        kt_sb = [persist.tile([128, ST], dt.bfloat16, tag=f"kt{i}", name=f"kt{i}")
                 for i in range(HP)]
        v_sb = [persist.tile([128, VW], dt.bfloat16, tag=f"v{i}", name=f"v{i}")
                for i in range(NST)]
        ctxn = [persist.tile([128, QROWS], dt.bfloat16, tag=f"ctxn{i}", name=f"ctxn{i}")
                for i in range(HP)]
        wo_sb = [persist.tile([128, DIM], dt.bfloat16, tag=f"wo{k}", name=f"wo{k}")
                 for k in range(HP)]
        mask_sb = [persist.tile([128, QROWS], dt.bfloat16, tag=f"mask{i}", name=f"mask{i}")
                   for i in range(NST)]
        bias_sb = persist.tile([128, HP + NK], dt.float32, tag="biases", name="biases")
        ones_col = persist.tile([128, 1], dt.bfloat16, tag="ones_col", name="ones_col")
        ones_row = persist.tile([1, 128], dt.bfloat16, tag="ones_row", name="ones_row")
        warm = persist.tile([1, 2], dt.float32, tag="warm", name="warm")

        projin_cm = tc.tile_pool(name="projin", bufs=1)
        projin = projin_cm.__enter__()
        xf_sb = [projin.tile([128, QROWS], dt.bfloat16, tag=f"xf{k}", name=f"xf{k}")
                 for k in range(NK)]
        xt_sb = [projin.tile([128, ST], dt.bfloat16, tag=f"xt{k}", name=f"xt{k}")
                 for k in range(NK)]
        wqkv_sb = [projin.tile([128, 3 * VW], dt.bfloat16, tag=f"wqkv{k}", name=f"wqkv{k}")
                   for k in range(NK)]
        wq_sb = [t[:, 0:VW] for t in wqkv_sb]
        wk_sb = [t[:, VW:2 * VW] for t in wqkv_sb]
        wv_sb = [t[:, 2 * VW:3 * VW] for t in wqkv_sb]

        # ---- DMA emission in first-use order (critical prologue set first) ----
        _ring = [0]

        def dma_crit(dst, src):
            eng = nc.sync if _ring[0] % 2 == 0 else nc.scalar
            _ring[0] += 1
            eng.dma_start(out=dst, in_=src)

        dma_crit(bias_sb, biases)
        for k in range(NK):
            dma_crit(wqkv_sb[k], wqkv[k * 128:(k + 1) * 128, :])
            dma_crit(xt_sb[k][:, 0:1024], xtT[k * 128:(k + 1) * 128, 0:1024])
        for k in range(NK):
            dma_crit(xf_sb[k][:, 0:1024], xfT[k * 128:(k + 1) * 128, 0:1024])
        for st in range(4):
            dma_crit(mask_sb[st], maskT[st * 128:(st + 1) * 128, :])
        # Bulk DMAs gated behind the first Q-projection write (all in-flight
        # DMAs share SDMA bandwidth round-robin; ungated bulk starves the
        # critical set). Demand-ordered across the two free rings: sync
        # carries the next masks (need ~1 tile/2.3us); gpsimd carries the
        # xt second half (v/kt rounds from gs 5), late masks, and wo.
        # The xf second half rides the SCALAR ring, emitted inside the exp
        # stream (gs 0-5) with no semaphore wait, so it self-throttles
        # behind the exps without blocking the ACT sequencer.
        bulk_sync = []
        for st in range(4, 9):
            bulk_sync.append((mask_sb[st], maskT[st * 128:(st + 1) * 128, :]))
        bulk_gps = []
        for k in range(NK):
            bulk_gps.append((xt_sb[k][:, 1024:ST], xtT[k * 128:(k + 1) * 128, 1024:ST]))
        for st in range(9, NST):
            bulk_gps.append((mask_sb[st], maskT[st * 128:(st + 1) * 128, :]))
        for k in range(HP):
            bulk_gps.append((wo_sb[k], wo[k * 128:(k + 1) * 128, :]))

        def emit_bulk_dmas():
            for dst, src in bulk_sync:
                nc.gpsimd.tensor_copy(out=dst[:, 0:1], in_=qt_sb[0][:, 0:1])
                nc.sync.dma_start(out=dst, in_=src)
            for dst, src in bulk_gps:
                nc.gpsimd.tensor_copy(out=dst[:, 0:1], in_=qt_sb[0][:, 0:1])
                nc.gpsimd.dma_start(out=dst, in_=src)

        nc.vector.memset(ones_col, 1.0)
        nc.vector.memset(ones_row, 1.0)

        # ---------------- projection rounds (all through psK, 1 bank) --------
        outsb_cm = tc.tile_pool(name="outsb", bufs=4)
        outsb = outsb_cm.__enter__()

        # qt/kt/v/o drains ride the DVE (ACT is the critical engine in-loop).
        # Rounds are generators yielding between half-lumps of ~3 matmuls so
        # no single lump between consecutive S-pairs exceeds ~750ns; the
        # driver pumps two half-lumps per step (before and after the ctx
        # emission), keeping the exp-feeding S matmuls on cadence.
        def qt_round(psK, hp, n0):
            ps = psK.tile([128, 512], dt.float32, tag="psK", name="psK")
            for k in range(NK):
                nc.tensor.matmul(ps, wq_sb[k][:, hp * 128:(hp + 1) * 128],
                                 xf_sb[k][:, n0:n0 + 512],
                                 start=(k == 0), stop=(k == NK - 1))
                if k == 2:
                    yield
            nc.vector.tensor_scalar_add(out=qt_sb[hp][:, n0:n0 + 512], in0=ps,
                                        scalar1=bias_sb[:, hp:hp + 1])

        def kt_round(psK, hp, n0):
            ps = psK.tile([128, 512], dt.float32, tag="psK", name="psK")
            for k in range(NK):
                nc.tensor.matmul(ps, wk_sb[k][:, hp * 128:(hp + 1) * 128],
                                 xt_sb[k][:, n0:n0 + 512],
                                 start=(k == 0), stop=(k == NK - 1))
                if k == 2:
                    yield
            nc.vector.tensor_copy(out=kt_sb[hp][:, n0:n0 + 512], in_=ps)

        def v_round(psK, st):
            ps = psK.tile([128, 512], dt.float32, tag="psK", name="psK")
            c0 = st * 128
            for k in range(NK):
                nc.tensor.matmul(ps[:, :VW], xt_sb[k][:, c0:c0 + 128], wv_sb[k],
                                 start=(k == 0), stop=(k == NK - 1))
                if k == 2:
                    yield
            nc.vector.tensor_copy(out=v_sb[st], in_=ps[:, :VW])

        def o_round(psK, of, n0, eng=None, drain_scalar=False):
            ps = psK.tile([128, 512], dt.float32, tag="psK", name="psK")
            for k in range(HP):
                nc.tensor.matmul(ps, wo_sb[k][:, of * 128:(of + 1) * 128],
                                 ctxn[k][:, n0:n0 + 512],
                                 start=(k == 0), stop=(k == HP - 1))
                if k == 1:
                    yield
            o = outsb.tile([128, 512], dt.bfloat16, tag="outsb", name="outsb")
            if drain_scalar:
                nc.scalar.activation(out=o, in_=ps,
                                     func=mybir.ActivationFunctionType.Identity,
                                     bias=bias_sb[:, HP + of:HP + of + 1])
            else:
                nc.vector.tensor_scalar_add(out=o, in0=ps,
                                            scalar1=bias_sb[:, HP + of:HP + of + 1])
            (eng or nc.gpsimd).dma_start(out=out[of * 128:(of + 1) * 128, n0:n0 + 512], in_=o)

        def run_full(gen):
            for _ in gen:
                pass

        # ---------------- static schedule: global step -> rounds --------
        # blocks: (0,q0) (0,q1) (1,q0) (2,q0) (1,q1) (2,q1); 16 steps each.
        # This order completes all qh=0 context by gs 63 so the 12 qh=0
        # output-projection rounds run in-loop from gs 70; only qh=1 O
        # rounds remain for the tail.
        sched = {}

        def at(gs, fn, *args):
            sched.setdefault(gs, []).append((fn,) + tuple(args))

        at(0, v_round, 2)
        at(0, v_round, 3)
        at(1, v_round, 4)
        at(1, v_round, 5)
        at(2, v_round, 6)
        at(2, v_round, 7)
        at(5, kt_round, 0, 1024)                  # xt-bulk gated; needed gs 8
        at(6, kt_round, 0, 1536)                  # needed gs 12
        for st in range(8, 13):                   # xt-bulk gated; v(st) by gs st
            at(st, v_round, st)
        at(13, v_round, 13)
        at(13, qt_round, 0, 1024)                 # xf via scalar ring; by gs 16
        at(13, qt_round, 0, 1536)
        at(14, v_round, 14)
        at(14, v_round, 15)
        for i, n0 in enumerate((0, 512, 1024, 1536)):  # KT[1] by gs 32
            at(17 + 2 * i, kt_round, 1, n0)
        at(25, qt_round, 1, 0)                    # QT[1] qh=0 by gs 32
        at(27, qt_round, 1, 512)
        for i, n0 in enumerate((0, 512, 1024, 1536)):  # KT[2] by gs 48
            at(33 + 2 * i, kt_round, 2, n0)
        at(41, qt_round, 2, 0)                    # QT[2] qh=0 by gs 48
        at(43, qt_round, 2, 512)
        at(49, qt_round, 1, 1024)                 # QT[1] qh=1 by gs 64
        at(51, qt_round, 1, 1536)
        at(65, qt_round, 2, 1024)                 # QT[2] qh=1 by gs 80
        at(67, qt_round, 2, 1536)
        # O rounds qh=0: ctxn[:, 0:1024] complete once block 3's deferred
        # normalize lands (~gs 66-68); 12 rounds at gs 70..81.
        gs_o = 70
        for n0 in (0, 512):
            for of in range(NK):
                at(gs_o, o_round, of, n0)
                gs_o += 1
        o_tail_extra = []

        # ---------------- attention ----------------
        ZJ = [(0, 0), (1, 0), (0, 512), (1, 512)]  # (h2, ni_off) per zps row 32j
        BLOCKS = [(0, 0), (0, 1), (1, 0), (2, 0), (1, 1), (2, 1)]

        with tc.tile_pool(name="attn", bufs=4) as attn, \
             tc.tile_pool(name="z97", bufs=1) as z97p, \
             tc.tile_pool(name="z2", bufs=1) as z2p, \
             tc.tile_pool(name="rzbc", bufs=1) as rzbcp, \
             tc.tile_pool(name="rzd", bufs=2, space="DRAM") as rzdp, \
             tc.tile_pool(name="psS", bufs=2, space="PSUM") as psS, \
             tc.tile_pool(name="psC", bufs=1, space="PSUM") as psC, \
             tc.tile_pool(name="psZ", bufs=1, space="PSUM") as psZ, \
             tc.tile_pool(name="psK", bufs=1, space="PSUM") as psK:

            # dummy exp so the ACT table loads during the prologue DMA wait
            nc.scalar.activation(out=warm, in_=bias_sb[0:1, 0:2], func=EXP)

            # PE warm-up: HAM clock-gate release before real work
            scratch = attn.tile([128, 1024], dt.bfloat16, tag="p", name="warmup_src")
            nc.vector.memset(scratch, 0.0)
            for i in range(12):
                ps = psS.tile([128, 1024], dt.float32, tag="sps", name="sps")
                nc.tensor.matmul(ps[:, 0:512], scratch[:, 0:128],
                                 scratch[:, 0:512], start=True, stop=True)

            # prologue compute (gated only on the critical DMA set). The
            # xt-gated rounds (kt, v) run during the ~6us-longer xf DMA
            # wait; rotating the psum bank across psK/psC/psZ lets each
            # round's matmuls overlap the previous round's DVE drain.
            run_full(kt_round(psK, 0, 0))
            pv0 = psC.tile([128, 1024], dt.float32, tag="ctxp", name="pv0")
            c0 = 0 * 128
            for k in range(NK):
                nc.tensor.matmul(pv0[:, 0:VW], xt_sb[k][:, 0:128], wv_sb[k],
                                 start=(k == 0), stop=(k == NK - 1))
            nc.vector.tensor_copy(out=v_sb[0], in_=pv0[:, 0:VW])
            pv1 = psZ.tile([128, 512], dt.float32, tag="zps", name="pv1")
            for k in range(NK):
                nc.tensor.matmul(pv1[:, 0:VW], xt_sb[k][:, 128:256], wv_sb[k],
                                 start=(k == 0), stop=(k == NK - 1))
            nc.vector.tensor_copy(out=v_sb[1], in_=pv1[:, 0:VW])
            run_full(kt_round(psK, 0, 512))       # needed gs 4
            run_full(qt_round(psK, 0, 0))
            run_full(qt_round(psK, 0, 512))
            emit_bulk_dmas()

            def emit_ctx(hp, qh, st, pp, ctxp, zps):
                # ctx pairs are emitted adjacently per ni so the col-packed
                # (0,0)/(0,64) matmuls run concurrently on the PE array;
                # the Z quad (col positions 0/32/64/96) likewise.
                for ni in range(2):
                    for h2 in (0, 1):
                        nc.tensor.matmul(
                            ctxp[64 * h2:64 * h2 + 64, 512 * ni:512 * ni + 512],
                            v_sb[st][:, (2 * hp + h2) * HD:(2 * hp + h2 + 1) * HD],
                            pp[ni][:, 512 * h2:512 * h2 + 512],
                            start=(st == 0), stop=(st == NST - 1),
                            tile_position=(0, 64 * h2))
                for j, (h2, noff) in enumerate(ZJ):
                    ni = noff // 512
                    nc.tensor.matmul(
                        zps[32 * j:32 * j + 1, 0:512],
                        ones_col,
                        pp[ni][:, 512 * h2:512 * h2 + 512],
                        start=(st == 0), stop=(st == NST - 1),
                        tile_position=(0, 32 * j))

            def drain_psum(hp, qh, ctxp, zps, last=False):
                # Free psZ (DVE reciprocal direct from PSUM) and psC (ACT
                # copy) quickly; the broadcast DMA chain then runs while the
                # deferred normalize waits on GpSimd.
                q0 = qh * 1024
                rz97 = z97p.tile([97, 512], dt.float32, tag="z97", name="z97")
                nc.vector.reciprocal_approx_fast(out=rz97, in_=zps[0:97, 0:512])
                cslice = ctxn[hp][:, q0:q0 + 1024]
                # ACT copy: psC released without queueing behind the DVE's
                # mask-multiply backlog
                nc.scalar.copy(out=cslice, in_=ctxp)
                rz97h = z97p.tile([97, 512], dt.bfloat16, tag="z97h", name="z97h")
                nc.vector.tensor_copy(out=rz97h, in_=rz97)
                if last:
                    # final block: 1/Z gathered onto ONE partition; the tail
                    # broadcasts it via a K=1 PE matmul (no DRAM bounce)
                    rzf = z2p.tile([1, 2048], dt.bfloat16, tag="rzf", name="rzf")
                    for j, (h2, noff) in enumerate(ZJ):
                        eng = nc.sync if j % 2 == 0 else nc.scalar
                        eng.dma_start(out=rzf[0:1, h2 * 1024 + noff:h2 * 1024 + noff + 512],
                                      in_=rz97h[32 * j:32 * j + 1, :])
                    return cslice, rzf
                rz2h = z2p.tile([2, 1024], dt.bfloat16, tag="rz2h", name="rz2h")
                for j, (h2, noff) in enumerate(ZJ):
                    nc.sync.dma_start(out=rz2h[h2:h2 + 1, noff:noff + 512],
                                      in_=rz97h[32 * j:32 * j + 1, :])
                rzd = rzdp.tile([2, 1024], dt.bfloat16, tag="rzd", name="rzd")
                nc.sync.dma_start(out=rzd, in_=rz2h)
                bc = rzbcp.tile([128, 1024], dt.bfloat16, tag="rzbc", name="rzbc")
                srcap = rzd[0:2, :]
                bcast = bass.AP(tensor=srcap.tensor, offset=srcap.offset,
                                ap=[srcap.ap[0], [0, HD], srcap.ap[1]])
                nc.sync.dma_start(out=bc, in_=bcast)
                return cslice, bc

            pending = None
            pending_norm = []
            ctxp_cur = None
            zps_cur = None
            round_q = []

            def pump(n):
                done = 0
                while round_q and done < n:
                    try:
                        next(round_q[0])
                        done += 1
                    except StopIteration:
                        round_q.pop(0)

            for bi, (hp, qh) in enumerate(BLOCKS):
                q0 = qh * 1024
                for st in range(NST):
                    gs = bi * NST + st
                    c0 = st * 128
                    if st == 0:
                        ctxp_cur = psC.tile([128, 1024], dt.float32,
                                            tag="ctxp", name="ctxp")
                        zps_cur = psZ.tile([128, 512], dt.float32,
                                           tag="zps", name="zps")
                    # S pairs: both ni emitted back-to-back so each
                    # row-packed (0,0)/(64,0) pair runs concurrently
                    sps_t = []
                    for ni in range(2):
                        n0 = q0 + 512 * ni
                        sps = psS.tile([128, 1024], dt.float32, tag="sps", name="sps")
                        for h2 in (0, 1):
                            nc.tensor.matmul(
                                sps[:, 512 * h2:512 * h2 + 512],
                                kt_sb[hp][HD * h2:HD * h2 + HD, c0:c0 + 128],
                                qt_sb[hp][HD * h2:HD * h2 + HD, n0:n0 + 512],
                                start=True, stop=True,
                                tile_position=(64 * h2, 0))
                        sps_t.append(sps)
                    pp = []
                    for ni in range(2):
                        n0 = q0 + 512 * ni
                        p = attn.tile([128, 1024], dt.bfloat16, tag="p", name="p")
                        nc.scalar.activation(out=p, in_=sps_t[ni], func=EXP)
                        m = mask_sb[st][:, n0:n0 + 512]
                        mrep = bass.AP(tensor=m.tensor, offset=m.offset,
                                       ap=[m.ap[0], [0, 2], m.ap[1]])
                        nc.vector.tensor_mul(out=p, in0=p, in1=mrep)
                        pp.append(p)
                    # xf second-half DMAs ride the ACT queue (scalar ring),
                    # self-throttled behind the early exps
                    if gs < NK:
                        nc.scalar.dma_start(
                            out=xf_sb[gs][:, 1024:QROWS],
                            in_=xfT[gs * 128:(gs + 1) * 128, 1024:QROWS])
                    # deferred 1/Z normalizes on GpSimd (waits there without
                    # head-of-line blocking the DVE)
                    while pending_norm and pending_norm[0][0] <= gs:
                        _, cs, bcx = pending_norm.pop(0)
                        nc.gpsimd.tensor_mul(out=cs, in0=cs, in1=bcx)
                    for entry in sched.get(gs, ()):
                        round_q.append(entry[0](psK, *entry[1:]))
                    # two pump points per step: one half-lump before the ctx
                    # emission, one after — except where block 0's schedule
                    # needs 2-3 rounds in one step
                    np_ = 2 if gs in (0, 1, 2, 13, 14, 15) else 1
                    pump(np_)
                    if pending is not None:
                        php, pqh, pst, ppp, pctxp, pzps = pending
                        emit_ctx(php, pqh, pst, ppp, pctxp, pzps)
                    pump(np_)
                    pending = (hp, qh, st, pp, ctxp_cur, zps_cur)
                    if st == NST - 1:
                        emit_ctx(hp, qh, st, pp, ctxp_cur, zps_cur)
                        cs, bcx = drain_psum(hp, qh, ctxp_cur, zps_cur,
                                             last=(bi == len(BLOCKS) - 1))
                        pending_norm.append((gs + 1, cs, bcx))
                        pending = None
            pump(1000)  # flush any unfinished rounds (should be none)
            assert len(pending_norm) == 1
            _, cs6, rzf6 = pending_norm.pop()

        # tail: output projection for query half 1 with 4 PSUM banks.
        # Software-pipelined: each o_round's hp0/hp1 partial matmuls (which
        # do not depend on the final block's normalize) run ahead — the
        # first two even before the 1/Z broadcast — so the PE never idles
        # on the normalize chain or on a drain.
        with tc.tile_pool(name="psO", bufs=4, space="PSUM") as psO:
            rlist = o_tail_extra + [(of, n0) for n0 in (1024, 1536)
                                    for of in range(NK)]
            gens = []
            for i, (of, n0) in enumerate(rlist):
                gens.append(o_round(psO, of, n0,
                                    eng=(nc.sync if i % 2 == 0 else nc.gpsimd),
                                    drain_scalar=(i % 2 == 1)))
            next(gens[0])
            next(gens[1])
            # finish the last block's 1/Z normalize via K=1 PE broadcast
            for qc in (0, 512):
                bcp = psO.tile([128, 512], dt.float32, tag="psK", name="bcp")
                for h2 in range(2):
                    nc.tensor.matmul(
                        bcp[64 * h2:64 * h2 + 64, :], ones_row[0:1, 0:64],
                        rzf6[0:1, h2 * 1024 + qc:h2 * 1024 + qc + 512],
                        start=True, stop=True, tile_position=(0, 64 * h2))
                nc.vector.tensor_mul(out=cs6[:, qc:qc + 512],
                                     in0=cs6[:, qc:qc + 512], in1=bcp)
            for i in range(len(gens)):
                run_full(gens[i])
                if i + 2 < len(gens):
                    next(gens[i + 2])

        outsb_cm.__exit__(None, None, None)
        projin_cm.__exit__(None, None, None)
        persist_cm.__exit__(None, None, None)

    nc.compile()
    return nc


def _get_nc():
    global _CACHED_NC
    if _CACHED_NC is None:
        _CACHED_NC = _build_nc()
    return _CACHED_NC


def _prep_inputs(from_tensor, to_tensor, attention_mask,
                 Wq, bq, Wk, bk, Wv, bv, Wo, bo):
    f32 = np.float32
    from_tensor = np.asarray(from_tensor, f32)
    to_tensor = np.asarray(to_tensor, f32)
    attention_mask = np.asarray(attention_mask)

    Wq, bq = np.asarray(Wq, f32), np.asarray(bq, f32)
    Wk = np.asarray(Wk, f32)
    Wv, bv = np.asarray(Wv, f32), np.asarray(bv, f32)
    Wo, bo = np.asarray(Wo, f32), np.asarray(bo, f32)
    wq_s = (Wq * SCALE).astype(BF16)
    wk_h = Wk.astype(BF16)
    wv_h = Wv.astype(BF16)

    xfT_all = [np.ascontiguousarray(from_tensor[b].T).astype(BF16) for b in range(B)]
    xtT_all = [np.ascontiguousarray(to_tensor[b].T).astype(BF16) for b in range(B)]
    maskT_all = [np.ascontiguousarray(attention_mask[b].T).astype(BF16)
                 for b in range(B)]

    per_g = []
    for g in range(2):
        i0 = g * VW
        wo_g = Wo[i0:i0 + VW, :]
        # bias folds: bk dropped (constant along softmax axis); bv folded into
        # the output bias (softmax weights sum to 1); bo added by g=0 only
        bo_eff = bv[i0:i0 + VW] @ wo_g + (bo if g == 0 else 0.0)
        biases = np.zeros((128, HP + NK), f32)
        biases[:, 0:HP] = (bq[i0:i0 + VW] * SCALE).reshape(HP, 128).T
        biases[:, HP:] = bo_eff.reshape(NK, 128).T
        per_g.append({
            "wqkv": np.ascontiguousarray(np.concatenate(
                [wq_s[:, i0:i0 + VW], wk_h[:, i0:i0 + VW],
                 wv_h[:, i0:i0 + VW]], axis=1)),
            "wo": wo_g.astype(BF16),
            "biases": biases,
        })

    in_maps = []
    for c in range(NCORES):
        b, g = c // 2, c % 2
        m = {"xfT": xfT_all[b], "xtT": xtT_all[b], "maskT": maskT_all[b]}
        m.update(per_g[g])
        in_maps.append(m)
    return in_maps


def _assemble(results):
    out = np.empty((B, SF, DIM), np.float32)
    for b in range(B):
        p0 = np.asarray(results[2 * b]["out"], np.float32)
        p1 = np.asarray(results[2 * b + 1]["out"], np.float32)
        out[b] = (p0 + p1).T
    return out


def _run(in_maps, trace=False):
    from concourse.bass_utils import run_bass_kernel_spmd
    nc = _get_nc()
    return run_bass_kernel_spmd(nc, in_maps, core_ids=list(range(NCORES)),
                                trace=trace)


def kernel(**inputs):
    in_maps = _prep_inputs(**inputs)
    res = _run(in_maps, trace=False)
    return _assemble(res.results)


def kernel_profiled(**inputs):
    """Returns (output, exec_time_ns, trace_path)."""
    in_maps = _prep_inputs(**inputs)
    res = _run(in_maps, trace=True)
    trace_path = None
    if res.instructions_and_trace is not None:
        trace_path = res.instructions_and_trace[1]
    return _assemble(res.results), res.exec_time_ns, trace_path


# revision 3
# speedup vs baseline: 1.0051x; 1.0051x over previous
"""Multi-head attention on 8 TRN2 NeuronCores — head-sharded, v3.

Problem: B=4, Sf=St=2048, DIM=768, H=12, Dh=64, f32 reference.

Sharding: core c = (batch b=c//2, head-group g=c%2); per-core 6 heads x all
2048 queries; host adds the two per-batch partials.

v3 changes vs v2 (the 368us baseline on current clocks):
  - ACT runs exps (the 1.2GHz-floor work) plus only the 6 ctx block drains;
    qt/kt round drains, o-round drains (in-loop), the 1/Z reciprocal and
    bf16 cast all move to the DVE, which has ~20% slack.
  - PE matmul emission is ordered so tile_position-packed groups sit
    adjacent in the PE queue (S pairs row-packed, ctx pairs col-packed,
    Z quads col-packed) — measured to execute concurrently (pair ~= 216ns).
  - v(0..7) projection rounds run in the prologue under the xf DMA wait;
    the in-loop schedule then stays at <= 1 round per step.
  - 10 of the 12 qh=0 output-projection rounds run inside block 5; the
    tail is the remaining 14 rounds with ACT drains (ACT is idle there).
"""

import numpy as np
import ml_dtypes

BF16 = ml_dtypes.bfloat16

B, SF, ST, DIM = 4, 2048, 2048, 768
NH, HD = 12, 64
SCALE = HD ** -0.5
NCORES = 8
NHC = NH // 2           # 6 heads per core
HP = NHC // 2           # 3 head-pairs per core
VW = NHC * HD           # 384: per-core inner width
QROWS = SF              # all 2048 query rows per core
NST = ST // 128         # 16 key tiles
NK = DIM // 128         # 6 contraction chunks

_CACHED_NC = None


def _build_nc():
    from concourse import bacc, tile, mybir
    import concourse.bass as bass

    dt = mybir.dt
    nc = bacc.Bacc("TRN2", target_bir_lowering=False, debug=False,
                   num_devices=NCORES)

    xfT = nc.dram_tensor("xfT", [DIM, QROWS], dt.bfloat16, kind="ExternalInput").ap()
    xtT = nc.dram_tensor("xtT", [DIM, ST], dt.bfloat16, kind="ExternalInput").ap()
    maskT = nc.dram_tensor("maskT", [ST, QROWS], dt.bfloat16, kind="ExternalInput").ap()
    wqkv = nc.dram_tensor("wqkv", [DIM, 3 * VW], dt.bfloat16, kind="ExternalInput").ap()
    wo = nc.dram_tensor("wo", [VW, DIM], dt.bfloat16, kind="ExternalInput").ap()
    biases = nc.dram_tensor("biases", [128, HP + NK], dt.float32, kind="ExternalInput").ap()
    out = nc.dram_tensor("out", [DIM, QROWS], dt.bfloat16, kind="ExternalOutput").ap()

    EXP = mybir.ActivationFunctionType.Exp

    with tile.TileContext(nc) as tc:
        persist_cm = tc.tile_pool(name="persist", bufs=1)
        persist = persist_cm.__enter__()

        qt_sb = [persist.tile([128, QROWS], dt.bfloat16, tag=f"qt{i}", name=f"qt{i}")
                 for i in range(HP)]
        kt_sb = [persist.tile([128, ST], dt.bfloat16, tag=f"kt{i}", name=f"kt{i}")
                 for i in range(HP)]
        v_sb = [persist.tile([128, VW], dt.bfloat16, tag=f"v{i}", name=f"v{i}")
                for i in range(NST)]
        ctxn = [persist.tile([128, QROWS], dt.bfloat16, tag=f"ctxn{i}", name=f"ctxn{i}")
                for i in range(HP)]
        wo_sb = [persist.tile([128, DIM], dt.bfloat16, tag=f"wo{k}", name=f"wo{k}")
                 for k in range(HP)]
        mask_sb = [persist.tile([128, QROWS], dt.bfloat16, tag=f"mask{i}", name=f"mask{i}")
                   for i in range(NST)]
        bias_sb = persist.tile([128, HP + NK], dt.float32, tag="biases", name="biases")
        ones_col = persist.tile([128, 1], dt.bfloat16, tag="ones_col", name="ones_col")
        ones_row = persist.tile([1, 128], dt.bfloat16, tag="ones_row", name="ones_row")
        warm = persist.tile([1, 2], dt.float32, tag="warm", name="warm")

        projin_cm = tc.tile_pool(name="projin", bufs=1)
        projin = projin_cm.__enter__()
        xf_sb = [projin.tile([128, QROWS], dt.bfloat16, tag=f"xf{k}", name=f"xf{k}")
                 for k in range(NK)]
        xt_sb = [projin.tile([128, ST], dt.bfloat16, tag=f"xt{k}", name=f"xt{k}")
                 for k in range(NK)]
        wqkv_sb = [projin.tile([128, 3 * VW], dt.bfloat16, tag=f"wqkv{k}", name=f"wqkv{k}")
                   for k in range(NK)]
        wq_sb = [t[:, 0:VW] for t in wqkv_sb]
        wk_sb = [t[:, VW:2 * VW] for t in wqkv_sb]
        wv_sb = [t[:, 2 * VW:3 * VW] for t in wqkv_sb]

        # ---- DMA emission in first-use order (critical prologue set first) ----
        _ring = [0]

        def dma_crit(dst, src):
            eng = nc.sync if _ring[0] % 2 == 0 else nc.scalar
            _ring[0] += 1
            eng.dma_start(out=dst, in_=src)

        dma_crit(bias_sb, biases)
        for k in range(NK):
            dma_crit(wqkv_sb[k], wqkv[k * 128:(k + 1) * 128, :])
            dma_crit(xt_sb[k][:, 0:1024], xtT[k * 128:(k + 1) * 128, 0:1024])
        for k in range(NK):
            dma_crit(xf_sb[k][:, 0:1024], xfT[k * 128:(k + 1) * 128, 0:1024])
        for st in range(4):
            dma_crit(mask_sb[st], maskT[st * 128:(st + 1) * 128, :])
        # Bulk DMAs gated behind the first Q-projection write (all in-flight
        # DMAs share SDMA bandwidth round-robin; ungated bulk starves the
        # critical set). Demand-ordered across the two free rings: sync
        # carries the next masks (need ~1 tile/2.3us); gpsimd carries the
        # xt second half (v/kt rounds from gs 5), late masks, and wo.
        # The xf second half rides the SCALAR ring, emitted inside the exp
        # stream (gs 0-5) with no semaphore wait, so it self-throttles
        # behind the exps without blocking the ACT sequencer.
        bulk_sync = []
        for st in range(4, 9):
            bulk_sync.append((mask_sb[st], maskT[st * 128:(st + 1) * 128, :]))
        bulk_gps = []
        for k in range(NK):
            bulk_gps.append((xt_sb[k][:, 1024:ST], xtT[k * 128:(k + 1) * 128, 1024:ST]))
        for st in range(9, NST):
            bulk_gps.append((mask_sb[st], maskT[st * 128:(st + 1) * 128, :]))
        for k in range(HP):
            bulk_gps.append((wo_sb[k], wo[k * 128:(k + 1) * 128, :]))

        def emit_bulk_dmas():
            for dst, src in bulk_sync:
                nc.gpsimd.tensor_copy(out=dst[:, 0:1], in_=qt_sb[0][:, 0:1])
                nc.sync.dma_start(out=dst, in_=src)
            for dst, src in bulk_gps:
                nc.gpsimd.tensor_copy(out=dst[:, 0:1], in_=qt_sb[0][:, 0:1])
                nc.gpsimd.dma_start(out=dst, in_=src)

        nc.vector.memset(ones_col, 1.0)
        nc.vector.memset(ones_row, 1.0)

        # ---------------- projection rounds (all through psK, 1 bank) --------
        outsb_cm = tc.tile_pool(name="outsb", bufs=4)
        outsb = outsb_cm.__enter__()

        # qt/kt/v/o drains ride the DVE (ACT is the critical engine in-loop).
        # Rounds are generators yielding between half-lumps of ~3 matmuls so
        # no single lump between consecutive S-pairs exceeds ~750ns; the
        # driver pumps two half-lumps per step (before and after the ctx
        # emission), keeping the exp-feeding S matmuls on cadence.
        def qt_round(psK, hp, n0):
            ps = psK.tile([128, 512], dt.float32, tag="psK", name="psK")
            for k in range(NK):
                nc.tensor.matmul(ps, wq_sb[k][:, hp * 128:(hp + 1) * 128],
                                 xf_sb[k][:, n0:n0 + 512],
                                 start=(k == 0), stop=(k == NK - 1))
                if k % 2 == 1 and k < NK - 1:
                    yield
            nc.vector.tensor_scalar_add(out=qt_sb[hp][:, n0:n0 + 512], in0=ps,
                                        scalar1=bias_sb[:, hp:hp + 1])

        def kt_round(psK, hp, n0):
            ps = psK.tile([128, 512], dt.float32, tag="psK", name="psK")
            for k in range(NK):
                nc.tensor.matmul(ps, wk_sb[k][:, hp * 128:(hp + 1) * 128],
                                 xt_sb[k][:, n0:n0 + 512],
                                 start=(k == 0), stop=(k == NK - 1))
                if k % 2 == 1 and k < NK - 1:
                    yield
            nc.vector.tensor_copy(out=kt_sb[hp][:, n0:n0 + 512], in_=ps)

        def v_round(psK, st):
            ps = psK.tile([128, 512], dt.float32, tag="psK", name="psK")
            c0 = st * 128
            for k in range(NK):
                nc.tensor.matmul(ps[:, :VW], xt_sb[k][:, c0:c0 + 128], wv_sb[k],
                                 start=(k == 0), stop=(k == NK - 1))
                if k % 2 == 1 and k < NK - 1:
                    yield
            nc.vector.tensor_copy(out=v_sb[st], in_=ps[:, :VW])

        def o_round(psK, of, n0, eng=None, drain_scalar=False):
            ps = psK.tile([128, 512], dt.float32, tag="psK", name="psK")
            for k in range(HP):
                nc.tensor.matmul(ps, wo_sb[k][:, of * 128:(of + 1) * 128],
                                 ctxn[k][:, n0:n0 + 512],
                                 start=(k == 0), stop=(k == HP - 1))
                if k == 1:
                    yield
            o = outsb.tile([128, 512], dt.bfloat16, tag="outsb", name="outsb")
            if drain_scalar:
                nc.scalar.activation(out=o, in_=ps,
                                     func=mybir.ActivationFunctionType.Identity,
                                     bias=bias_sb[:, HP + of:HP + of + 1])
            else:
                nc.vector.tensor_scalar_add(out=o, in0=ps,
                                            scalar1=bias_sb[:, HP + of:HP + of + 1])
            (eng or nc.gpsimd).dma_start(out=out[of * 128:(of + 1) * 128, n0:n0 + 512], in_=o)

        def run_full(gen):
            for _ in gen:
                pass

        # ---------------- static schedule: global step -> rounds --------
        # blocks: (0,q0) (0,q1) (1,q0) (2,q0) (1,q1) (2,q1); 16 steps each.
        # This order completes all qh=0 context by gs 63 so the 12 qh=0
        # output-projection rounds run in-loop from gs 70; only qh=1 O
        # rounds remain for the tail.
        sched = {}

        def at(gs, fn, *args):
            sched.setdefault(gs, []).append((fn,) + tuple(args))

        at(0, v_round, 2)
        at(0, v_round, 3)
        at(1, v_round, 4)
        at(1, v_round, 5)
        at(2, v_round, 6)
        at(2, v_round, 7)
        at(5, kt_round, 0, 1024)                  # xt-bulk gated; needed gs 8
        at(6, kt_round, 0, 1536)                  # needed gs 12
        for st in range(8, 13):                   # xt-bulk gated; v(st) by gs st
            at(st, v_round, st)
        at(13, v_round, 13)
        at(13, qt_round, 0, 1024)                 # xf via scalar ring; by gs 16
        at(13, qt_round, 0, 1536)
        at(14, v_round, 14)
        at(14, v_round, 15)
        for i, n0 in enumerate((0, 512, 1024, 1536)):  # KT[1] by gs 32
            at(17 + 2 * i, kt_round, 1, n0)
        at(25, qt_round, 1, 0)                    # QT[1] qh=0 by gs 32
        at(27, qt_round, 1, 512)
        for i, n0 in enumerate((0, 512, 1024, 1536)):  # KT[2] by gs 48
            at(33 + 2 * i, kt_round, 2, n0)
        at(41, qt_round, 2, 0)                    # QT[2] qh=0 by gs 48
        at(43, qt_round, 2, 512)
        at(49, qt_round, 1, 1024)                 # QT[1] qh=1 by gs 64
        at(51, qt_round, 1, 1536)
        at(65, qt_round, 2, 1024)                 # QT[2] qh=1 by gs 80
        at(67, qt_round, 2, 1536)
        # O rounds qh=0: ctxn[:, 0:1024] complete once block 3's deferred
        # normalize lands (~gs 66-68); 12 rounds at gs 70..81.
        gs_o = 70
        for n0 in (0, 512):
            for of in range(NK):
                at(gs_o, o_round, of, n0)
                gs_o += 1
        o_tail_extra = []

        # ---------------- attention ----------------
        ZJ = [(0, 0), (1, 0), (0, 512), (1, 512)]  # (h2, ni_off) per zps row 32j
        BLOCKS = [(0, 0), (0, 1), (1, 0), (2, 0), (1, 1), (2, 1)]

        with tc.tile_pool(name="attn", bufs=4) as attn, \
             tc.tile_pool(name="z97", bufs=1) as z97p, \
             tc.tile_pool(name="z2", bufs=1) as z2p, \
             tc.tile_pool(name="rzbc", bufs=1) as rzbcp, \
             tc.tile_pool(name="rzd", bufs=2, space="DRAM") as rzdp, \
             tc.tile_pool(name="psS", bufs=2, space="PSUM") as psS, \
             tc.tile_pool(name="psC", bufs=1, space="PSUM") as psC, \
             tc.tile_pool(name="psZ", bufs=1, space="PSUM") as psZ, \
             tc.tile_pool(name="psK", bufs=1, space="PSUM") as psK:

            # dummy exp so the ACT table loads during the prologue DMA wait
            nc.scalar.activation(out=warm, in_=bias_sb[0:1, 0:2], func=EXP)

            # PE warm-up: HAM clock-gate release before real work
            scratch = attn.tile([128, 1024], dt.bfloat16, tag="p", name="warmup_src")
            nc.vector.memset(scratch, 0.0)
            for i in range(12):
                ps = psS.tile([128, 1024], dt.float32, tag="sps", name="sps")
                nc.tensor.matmul(ps[:, 0:512], scratch[:, 0:128],
                                 scratch[:, 0:512], start=True, stop=True)

            # prologue compute (gated only on the critical DMA set). The
            # xt-gated rounds (kt, v) run during the ~6us-longer xf DMA
            # wait; rotating the psum bank across psK/psC/psZ lets each
            # round's matmuls overlap the previous round's DVE drain.
            run_full(kt_round(psK, 0, 0))
            pv0 = psC.tile([128, 1024], dt.float32, tag="ctxp", name="pv0")
            c0 = 0 * 128
            for k in range(NK):
                nc.tensor.matmul(pv0[:, 0:VW], xt_sb[k][:, 0:128], wv_sb[k],
                                 start=(k == 0), stop=(k == NK - 1))
            nc.vector.tensor_copy(out=v_sb[0], in_=pv0[:, 0:VW])
            pv1 = psZ.tile([128, 512], dt.float32, tag="zps", name="pv1")
            for k in range(NK):
                nc.tensor.matmul(pv1[:, 0:VW], xt_sb[k][:, 128:256], wv_sb[k],
                                 start=(k == 0), stop=(k == NK - 1))
            nc.vector.tensor_copy(out=v_sb[1], in_=pv1[:, 0:VW])
            run_full(kt_round(psK, 0, 512))       # needed gs 4
            run_full(qt_round(psK, 0, 0))
            run_full(qt_round(psK, 0, 512))
            emit_bulk_dmas()

            def emit_ctx(hp, qh, st, pp, ctxp, zps):
                # ctx pairs are emitted adjacently per ni so the col-packed
                # (0,0)/(0,64) matmuls run concurrently on the PE array;
                # the Z quad (col positions 0/32/64/96) likewise.
                for ni in range(2):
                    for h2 in (0, 1):
                        nc.tensor.matmul(
                            ctxp[64 * h2:64 * h2 + 64, 512 * ni:512 * ni + 512],
                            v_sb[st][:, (2 * hp + h2) * HD:(2 * hp + h2 + 1) * HD],
                            pp[ni][:, 512 * h2:512 * h2 + 512],
                            start=(st == 0), stop=(st == NST - 1),
                            tile_position=(0, 64 * h2))
                for j, (h2, noff) in enumerate(ZJ):
                    ni = noff // 512
                    nc.tensor.matmul(
                        zps[32 * j:32 * j + 1, 0:512],
                        ones_col,
                        pp[ni][:, 512 * h2:512 * h2 + 512],
                        start=(st == 0), stop=(st == NST - 1),
                        tile_position=(0, 32 * j))

            def drain_psum(hp, qh, ctxp, zps, last=False):
                # Free psZ (DVE reciprocal direct from PSUM) and psC (ACT
                # copy) quickly; the broadcast DMA chain then runs while the
                # deferred normalize waits on GpSimd.
                q0 = qh * 1024
                rz97 = z97p.tile([97, 512], dt.float32, tag="z97", name="z97")
                nc.vector.reciprocal_approx_fast(out=rz97, in_=zps[0:97, 0:512])
                cslice = ctxn[hp][:, q0:q0 + 1024]
                # ACT copy: psC released without queueing behind the DVE's
                # mask-multiply backlog
                nc.scalar.copy(out=cslice, in_=ctxp)
                rz97h = z97p.tile([97, 512], dt.bfloat16, tag="z97h", name="z97h")
                nc.vector.tensor_copy(out=rz97h, in_=rz97)
                if last:
                    # final block: 1/Z gathered onto ONE partition; the tail
                    # broadcasts it via a K=1 PE matmul (no DRAM bounce)
                    rzf = z2p.tile([1, 2048], dt.bfloat16, tag="rzf", name="rzf")
                    for j, (h2, noff) in enumerate(ZJ):
                        eng = nc.sync if j % 2 == 0 else nc.scalar
                        eng.dma_start(out=rzf[0:1, h2 * 1024 + noff:h2 * 1024 + noff + 512],
                                      in_=rz97h[32 * j:32 * j + 1, :])
                    return cslice, rzf
                rz2h = z2p.tile([2, 1024], dt.bfloat16, tag="rz2h", name="rz2h")
                for j, (h2, noff) in enumerate(ZJ):
                    nc.sync.dma_start(out=rz2h[h2:h2 + 1, noff:noff + 512],
                                      in_=rz97h[32 * j:32 * j + 1, :])
                rzd = rzdp.tile([2, 1024], dt.bfloat16, tag="rzd", name="rzd")
                nc.sync.dma_start(out=rzd, in_=rz2h)
                bc = rzbcp.tile([128, 1024], dt.bfloat16, tag="rzbc", name="rzbc")
                srcap = rzd[0:2, :]
                bcast = bass.AP(tensor=srcap.tensor, offset=srcap.offset,
                                ap=[srcap.ap[0], [0, HD], srcap.ap[1]])
                nc.sync.dma_start(out=bc, in_=bcast)
                return cslice, bc

            pending = None
            pending_norm = []
            ctxp_cur = None
            zps_cur = None
            round_q = []

            def pump(n):
                done = 0
                while round_q and done < n:
                    try:
                        next(round_q[0])
                        done += 1
                    except StopIteration:
                        round_q.pop(0)

            for bi, (hp, qh) in enumerate(BLOCKS):
                q0 = qh * 1024
                for st in range(NST):
                    gs = bi * NST + st
                    c0 = st * 128
                    if st == 0:
                        ctxp_cur = psC.tile([128, 1024], dt.float32,
                                            tag="ctxp", name="ctxp")
                        zps_cur = psZ.tile([128, 512], dt.float32,
                                           tag="zps", name="zps")
                    # S pairs: both ni emitted back-to-back so each
                    # row-packed (0,0)/(64,0) pair runs concurrently
                    sps_t = []
                    for ni in range(2):
                        n0 = q0 + 512 * ni
                        sps = psS.tile([128, 1024], dt.float32, tag="sps", name="sps")
                        for h2 in (0, 1):
                            nc.tensor.matmul(
                                sps[:, 512 * h2:512 * h2 + 512],
                                kt_sb[hp][HD * h2:HD * h2 + HD, c0:c0 + 128],
                                qt_sb[hp][HD * h2:HD * h2 + HD, n0:n0 + 512],
                                start=True, stop=True,
                                tile_position=(64 * h2, 0))
                        sps_t.append(sps)
                    pp = []
                    for ni in range(2):
                        n0 = q0 + 512 * ni
                        p = attn.tile([128, 1024], dt.bfloat16, tag="p", name="p")
                        nc.scalar.activation(out=p, in_=sps_t[ni], func=EXP)
                        m = mask_sb[st][:, n0:n0 + 512]
                        mrep = bass.AP(tensor=m.tensor, offset=m.offset,
                                       ap=[m.ap[0], [0, 2], m.ap[1]])
                        nc.vector.tensor_mul(out=p, in0=p, in1=mrep)
                        pp.append(p)
                    # xf second-half DMAs ride the ACT queue (scalar ring),
                    # self-throttled behind the early exps
                    if gs < NK:
                        nc.scalar.dma_start(
                            out=xf_sb[gs][:, 1024:QROWS],
                            in_=xfT[gs * 128:(gs + 1) * 128, 1024:QROWS])
                    # deferred 1/Z normalizes on GpSimd (waits there without
                    # head-of-line blocking the DVE)
                    while pending_norm and pending_norm[0][0] <= gs:
                        _, cs, bcx = pending_norm.pop(0)
                        nc.gpsimd.tensor_mul(out=cs, in0=cs, in1=bcx)
                    for entry in sched.get(gs, ()):
                        round_q.append(entry[0](psK, *entry[1:]))
                    # two pump points per step: one half-lump before the ctx
                    # emission, one after — except where block 0's schedule
                    # needs 2-3 rounds in one step
                    np_ = 3 if gs in (0, 1, 2, 13, 14, 15) else 1
                    pump(np_)
                    if pending is not None:
                        php, pqh, pst, ppp, pctxp, pzps = pending
                        emit_ctx(php, pqh, pst, ppp, pctxp, pzps)
                    pump(np_)
                    pending = (hp, qh, st, pp, ctxp_cur, zps_cur)
                    if st == NST - 1:
                        emit_ctx(hp, qh, st, pp, ctxp_cur, zps_cur)
                        cs, bcx = drain_psum(hp, qh, ctxp_cur, zps_cur,
                                             last=(bi == len(BLOCKS) - 1))
                        pending_norm.append((gs + 1, cs, bcx))
                        pending = None
            pump(1000)  # flush any unfinished rounds (should be none)
            assert len(pending_norm) == 1
            _, cs6, rzf6 = pending_norm.pop()

        # tail: output projection for query half 1 with 4 PSUM banks.
        # Software-pipelined: each o_round's hp0/hp1 partial matmuls (which
        # do not depend on the final block's normalize) run ahead — the
        # first two even before the 1/Z broadcast — so the PE never idles
        # on the normalize chain or on a drain.
        with tc.tile_pool(name="psO", bufs=4, space="PSUM") as psO:
            rlist = o_tail_extra + [(of, n0) for n0 in (1024, 1536)
                                    for of in range(NK)]
            gens = []
            for i, (of, n0) in enumerate(rlist):
                gens.append(o_round(psO, of, n0,
                                    eng=(nc.sync if i % 2 == 0 else nc.gpsimd),
                                    drain_scalar=(i % 2 == 1)))
            next(gens[0])
            next(gens[1])
            # finish the last block's 1/Z normalize via K=1 PE broadcast
            for qc in (0, 512):
                bcp = psO.tile([128, 512], dt.float32, tag="psK", name="bcp")
                for h2 in range(2):
                    nc.tensor.matmul(
                        bcp[64 * h2:64 * h2 + 64, :], ones_row[0:1, 0:64],
                        rzf6[0:1, h2 * 1024 + qc:h2 * 1024 + qc + 512],
                        start=True, stop=True, tile_position=(0, 64 * h2))
                nc.vector.tensor_mul(out=cs6[:, qc:qc + 512],
                                     in0=cs6[:, qc:qc + 512], in1=bcp)
            for i in range(len(gens)):
                run_full(gens[i])
                if i + 2 < len(gens):
                    next(gens[i + 2])

        outsb_cm.__exit__(None, None, None)
        projin_cm.__exit__(None, None, None)
        persist_cm.__exit__(None, None, None)

    nc.compile()
    return nc


def _get_nc():
    global _CACHED_NC
    if _CACHED_NC is None:
        _CACHED_NC = _build_nc()
    return _CACHED_NC


def _prep_inputs(from_tensor, to_tensor, attention_mask,
                 Wq, bq, Wk, bk, Wv, bv, Wo, bo):
    f32 = np.float32
    from_tensor = np.asarray(from_tensor, f32)
    to_tensor = np.asarray(to_tensor, f32)
    attention_mask = np.asarray(attention_mask)

    Wq, bq = np.asarray(Wq, f32), np.asarray(bq, f32)
    Wk = np.asarray(Wk, f32)
    Wv, bv = np.asarray(Wv, f32), np.asarray(bv, f32)
    Wo, bo = np.asarray(Wo, f32), np.asarray(bo, f32)
    wq_s = (Wq * SCALE).astype(BF16)
    wk_h = Wk.astype(BF16)
    wv_h = Wv.astype(BF16)

    xfT_all = [np.ascontiguousarray(from_tensor[b].T).astype(BF16) for b in range(B)]
    xtT_all = [np.ascontiguousarray(to_tensor[b].T).astype(BF16) for b in range(B)]
    maskT_all = [np.ascontiguousarray(attention_mask[b].T).astype(BF16)
                 for b in range(B)]

    per_g = []
    for g in range(2):
        i0 = g * VW
        wo_g = Wo[i0:i0 + VW, :]
        # bias folds: bk dropped (constant along softmax axis); bv folded into
        # the output bias (softmax weights sum to 1); bo added by g=0 only
        bo_eff = bv[i0:i0 + VW] @ wo_g + (bo if g == 0 else 0.0)
        biases = np.zeros((128, HP + NK), f32)
        biases[:, 0:HP] = (bq[i0:i0 + VW] * SCALE).reshape(HP, 128).T
        biases[:, HP:] = bo_eff.reshape(NK, 128).T
        per_g.append({
            "wqkv": np.ascontiguousarray(np.concatenate(
                [wq_s[:, i0:i0 + VW], wk_h[:, i0:i0 + VW],
                 wv_h[:, i0:i0 + VW]], axis=1)),
            "wo": wo_g.astype(BF16),
            "biases": biases,
        })

    in_maps = []
    for c in range(NCORES):
        b, g = c // 2, c % 2
        m = {"xfT": xfT_all[b], "xtT": xtT_all[b], "maskT": maskT_all[b]}
        m.update(per_g[g])
        in_maps.append(m)
    return in_maps


def _assemble(results):
    out = np.empty((B, SF, DIM), np.float32)
    for b in range(B):
        p0 = np.asarray(results[2 * b]["out"], np.float32)
        p1 = np.asarray(results[2 * b + 1]["out"], np.float32)
        out[b] = (p0 + p1).T
    return out


def _run(in_maps, trace=False):
    from concourse.bass_utils import run_bass_kernel_spmd
    nc = _get_nc()
    return run_bass_kernel_spmd(nc, in_maps, core_ids=list(range(NCORES)),
                                trace=trace)


def kernel(**inputs):
    in_maps = _prep_inputs(**inputs)
    res = _run(in_maps, trace=False)
    return _assemble(res.results)


def kernel_profiled(**inputs):
    """Returns (output, exec_time_ns, trace_path)."""
    in_maps = _prep_inputs(**inputs)
    res = _run(in_maps, trace=True)
    trace_path = None
    if res.instructions_and_trace is not None:
        trace_path = res.instructions_and_trace[1]
    return _assemble(res.results), res.exec_time_ns, trace_path


# revision 4
# speedup vs baseline: 1.0148x; 1.0096x over previous
"""Multi-head attention on 8 TRN2 NeuronCores — head-sharded, v3.

Problem: B=4, Sf=St=2048, DIM=768, H=12, Dh=64, f32 reference.

Sharding: core c = (batch b=c//2, head-group g=c%2); per-core 6 heads x all
2048 queries; host adds the two per-batch partials.

v3 changes vs v2 (the 368us baseline on current clocks):
  - ACT runs exps (the 1.2GHz-floor work) plus only the 6 ctx block drains;
    qt/kt round drains, o-round drains (in-loop), the 1/Z reciprocal and
    bf16 cast all move to the DVE, which has ~20% slack.
  - PE matmul emission is ordered so tile_position-packed groups sit
    adjacent in the PE queue (S pairs row-packed, ctx pairs col-packed,
    Z quads col-packed) — measured to execute concurrently (pair ~= 216ns).
  - v(0..7) projection rounds run in the prologue under the xf DMA wait;
    the in-loop schedule then stays at <= 1 round per step.
  - 10 of the 12 qh=0 output-projection rounds run inside block 5; the
    tail is the remaining 14 rounds with ACT drains (ACT is idle there).
"""

import numpy as np
import ml_dtypes

BF16 = ml_dtypes.bfloat16

B, SF, ST, DIM = 4, 2048, 2048, 768
NH, HD = 12, 64
SCALE = HD ** -0.5
NCORES = 8
NHC = NH // 2           # 6 heads per core
HP = NHC // 2           # 3 head-pairs per core
VW = NHC * HD           # 384: per-core inner width
QROWS = SF              # all 2048 query rows per core
NST = ST // 128         # 16 key tiles
NK = DIM // 128         # 6 contraction chunks

_CACHED_NC = None


def _build_nc():
    from concourse import bacc, tile, mybir
    import concourse.bass as bass

    dt = mybir.dt
    nc = bacc.Bacc("TRN2", target_bir_lowering=False, debug=False,
                   num_devices=NCORES)

    xfT = nc.dram_tensor("xfT", [DIM, QROWS], dt.bfloat16, kind="ExternalInput").ap()
    xtT = nc.dram_tensor("xtT", [DIM, ST], dt.bfloat16, kind="ExternalInput").ap()
    maskT = nc.dram_tensor("maskT", [ST, QROWS], dt.bfloat16, kind="ExternalInput").ap()
    wqkv = nc.dram_tensor("wqkv", [DIM, 3 * VW], dt.bfloat16, kind="ExternalInput").ap()
    wo = nc.dram_tensor("wo", [VW, DIM], dt.bfloat16, kind="ExternalInput").ap()
    biases = nc.dram_tensor("biases", [128, HP + NK], dt.float32, kind="ExternalInput").ap()
    out = nc.dram_tensor("out", [DIM, QROWS], dt.bfloat16, kind="ExternalOutput").ap()

    EXP = mybir.ActivationFunctionType.Exp

    with tile.TileContext(nc) as tc:
        persist_cm = tc.tile_pool(name="persist", bufs=1)
        persist = persist_cm.__enter__()

        qt_sb = [persist.tile([128, QROWS], dt.bfloat16, tag=f"qt{i}", name=f"qt{i}")
                 for i in range(HP)]
        kt_sb = [persist.tile([128, ST], dt.bfloat16, tag=f"kt{i}", name=f"kt{i}")
                 for i in range(HP)]
        v_sb = [persist.tile([128, VW], dt.bfloat16, tag=f"v{i}", name=f"v{i}")
                for i in range(NST)]
        ctxn = [persist.tile([128, QROWS], dt.bfloat16, tag=f"ctxn{i}", name=f"ctxn{i}")
                for i in range(HP)]
        wo_sb = [persist.tile([128, DIM], dt.bfloat16, tag=f"wo{k}", name=f"wo{k}")
                 for k in range(HP)]
        mask_sb = [persist.tile([128, QROWS], dt.bfloat16, tag=f"mask{i}", name=f"mask{i}")
                   for i in range(NST)]
        bias_sb = persist.tile([128, HP + NK], dt.float32, tag="biases", name="biases")
        ones_col = persist.tile([128, 1], dt.bfloat16, tag="ones_col", name="ones_col")
        ones_row = persist.tile([1, 128], dt.bfloat16, tag="ones_row", name="ones_row")
        warm = persist.tile([1, 2], dt.float32, tag="warm", name="warm")

        projin_cm = tc.tile_pool(name="projin", bufs=1)
        projin = projin_cm.__enter__()
        xf_sb = [projin.tile([128, QROWS], dt.bfloat16, tag=f"xf{k}", name=f"xf{k}")
                 for k in range(NK)]
        xt_sb = [projin.tile([128, ST], dt.bfloat16, tag=f"xt{k}", name=f"xt{k}")
                 for k in range(NK)]
        wqkv_sb = [projin.tile([128, 3 * VW], dt.bfloat16, tag=f"wqkv{k}", name=f"wqkv{k}")
                   for k in range(NK)]
        wq_sb = [t[:, 0:VW] for t in wqkv_sb]
        wk_sb = [t[:, VW:2 * VW] for t in wqkv_sb]
        wv_sb = [t[:, 2 * VW:3 * VW] for t in wqkv_sb]

        # ---- DMA emission in first-use order (critical prologue set first) ----
        _ring = [0]

        def dma_crit(dst, src):
            eng = nc.sync if _ring[0] % 2 == 0 else nc.scalar
            _ring[0] += 1
            eng.dma_start(out=dst, in_=src)

        dma_crit(bias_sb, biases)
        for k in range(NK):
            dma_crit(wqkv_sb[k], wqkv[k * 128:(k + 1) * 128, :])
            dma_crit(xt_sb[k][:, 0:1024], xtT[k * 128:(k + 1) * 128, 0:1024])
        for k in range(NK):
            dma_crit(xf_sb[k][:, 0:1024], xfT[k * 128:(k + 1) * 128, 0:1024])
        for st in range(4):
            dma_crit(mask_sb[st][:, 0:1024], maskT[st * 128:(st + 1) * 128, 0:1024])
        # Bulk DMAs gated behind the first Q-projection write (all in-flight
        # DMAs share SDMA bandwidth round-robin; ungated bulk starves the
        # critical set). Demand-ordered across the two free rings: sync
        # carries the next masks (need ~1 tile/2.3us); gpsimd carries the
        # xt second half (v/kt rounds from gs 5), late masks, and wo.
        # The xf second half rides the SCALAR ring, emitted inside the exp
        # stream (gs 0-5) with no semaphore wait, so it self-throttles
        # behind the exps without blocking the ACT sequencer.
        bulk_sync = []
        for st in range(4, 9):
            bulk_sync.append((mask_sb[st][:, 0:1024],
                              maskT[st * 128:(st + 1) * 128, 0:1024]))
        bulk_gps = []
        for k in range(NK):
            bulk_gps.append((xt_sb[k][:, 1024:ST], xtT[k * 128:(k + 1) * 128, 1024:ST]))
        for st in range(9, NST):
            bulk_gps.append((mask_sb[st], maskT[st * 128:(st + 1) * 128, :]))
        for st in range(9):
            # query-half-1 mask columns: first consumed at gs 16
            bulk_gps.append((mask_sb[st][:, 1024:2048],
                             maskT[st * 128:(st + 1) * 128, 1024:2048]))
        for k in range(HP):
            bulk_gps.append((wo_sb[k], wo[k * 128:(k + 1) * 128, :]))

        def emit_bulk_dmas():
            for dst, src in bulk_sync:
                nc.gpsimd.tensor_copy(out=dst[:, 0:1], in_=qt_sb[0][:, 0:1])
                nc.sync.dma_start(out=dst, in_=src)
            for dst, src in bulk_gps:
                nc.gpsimd.tensor_copy(out=dst[:, 0:1], in_=qt_sb[0][:, 0:1])
                nc.gpsimd.dma_start(out=dst, in_=src)

        nc.vector.memset(ones_col, 1.0)
        nc.vector.memset(ones_row, 1.0)

        # ---------------- projection rounds (all through psK, 1 bank) --------
        outsb_cm = tc.tile_pool(name="outsb", bufs=4)
        outsb = outsb_cm.__enter__()

        # qt/kt/v/o drains ride the DVE (ACT is the critical engine in-loop).
        # Rounds are generators yielding between half-lumps of ~3 matmuls so
        # no single lump between consecutive S-pairs exceeds ~750ns; the
        # driver pumps two half-lumps per step (before and after the ctx
        # emission), keeping the exp-feeding S matmuls on cadence.
        def qt_round(psK, hp, n0):
            ps = psK.tile([128, 512], dt.float32, tag="psK", name="psK")
            for k in range(NK):
                nc.tensor.matmul(ps, wq_sb[k][:, hp * 128:(hp + 1) * 128],
                                 xf_sb[k][:, n0:n0 + 512],
                                 start=(k == 0), stop=(k == NK - 1))
                if k % 2 == 1 and k < NK - 1:
                    yield
            nc.vector.tensor_scalar_add(out=qt_sb[hp][:, n0:n0 + 512], in0=ps,
                                        scalar1=bias_sb[:, hp:hp + 1])

        def kt_round(psK, hp, n0):
            ps = psK.tile([128, 512], dt.float32, tag="psK", name="psK")
            for k in range(NK):
                nc.tensor.matmul(ps, wk_sb[k][:, hp * 128:(hp + 1) * 128],
                                 xt_sb[k][:, n0:n0 + 512],
                                 start=(k == 0), stop=(k == NK - 1))
                if k % 2 == 1 and k < NK - 1:
                    yield
            nc.vector.tensor_copy(out=kt_sb[hp][:, n0:n0 + 512], in_=ps)

        def v_round(psK, st):
            ps = psK.tile([128, 512], dt.float32, tag="psK", name="psK")
            c0 = st * 128
            for k in range(NK):
                nc.tensor.matmul(ps[:, :VW], xt_sb[k][:, c0:c0 + 128], wv_sb[k],
                                 start=(k == 0), stop=(k == NK - 1))
                if k % 2 == 1 and k < NK - 1:
                    yield
            nc.vector.tensor_copy(out=v_sb[st], in_=ps[:, :VW])

        def o_round(psK, of, n0, eng=None, drain_scalar=False):
            ps = psK.tile([128, 512], dt.float32, tag="psK", name="psK")
            for k in range(HP):
                nc.tensor.matmul(ps, wo_sb[k][:, of * 128:(of + 1) * 128],
                                 ctxn[k][:, n0:n0 + 512],
                                 start=(k == 0), stop=(k == HP - 1))
                if k == 1:
                    yield
            o = outsb.tile([128, 512], dt.bfloat16, tag="outsb", name="outsb")
            if drain_scalar:
                nc.scalar.activation(out=o, in_=ps,
                                     func=mybir.ActivationFunctionType.Identity,
                                     bias=bias_sb[:, HP + of:HP + of + 1])
            else:
                nc.vector.tensor_scalar_add(out=o, in0=ps,
                                            scalar1=bias_sb[:, HP + of:HP + of + 1])
            (eng or nc.gpsimd).dma_start(out=out[of * 128:(of + 1) * 128, n0:n0 + 512], in_=o)

        def run_full(gen):
            for _ in gen:
                pass

        # ---------------- static schedule: global step -> rounds --------
        # blocks: (0,q0) (0,q1) (1,q0) (2,q0) (1,q1) (2,q1); 16 steps each.
        # This order completes all qh=0 context by gs 63 so the 12 qh=0
        # output-projection rounds run in-loop from gs 70; only qh=1 O
        # rounds remain for the tail.
        sched = {}

        def at(gs, fn, *args):
            sched.setdefault(gs, []).append((fn,) + tuple(args))

        at(0, v_round, 2)
        at(0, v_round, 3)
        at(1, v_round, 4)
        at(1, v_round, 5)
        at(2, v_round, 6)
        at(2, v_round, 7)
        at(5, kt_round, 0, 1024)                  # xt-bulk gated; needed gs 8
        at(6, kt_round, 0, 1536)                  # needed gs 12
        for st in range(8, 13):                   # xt-bulk gated; v(st) by gs st
            at(st, v_round, st)
        at(13, v_round, 13)
        at(13, qt_round, 0, 1024)                 # xf via scalar ring; by gs 16
        at(13, qt_round, 0, 1536)
        at(14, v_round, 14)
        at(14, v_round, 15)
        for i, n0 in enumerate((0, 512, 1024, 1536)):  # KT[1] by gs 32
            at(17 + 2 * i, kt_round, 1, n0)
        at(25, qt_round, 1, 0)                    # QT[1] qh=0 by gs 32
        at(27, qt_round, 1, 512)
        for i, n0 in enumerate((0, 512, 1024, 1536)):  # KT[2] by gs 48
            at(33 + 2 * i, kt_round, 2, n0)
        at(41, qt_round, 2, 0)                    # QT[2] qh=0 by gs 48
        at(43, qt_round, 2, 512)
        at(49, qt_round, 1, 1024)                 # QT[1] qh=1 by gs 64
        at(51, qt_round, 1, 1536)
        at(65, qt_round, 2, 1024)                 # QT[2] qh=1 by gs 80
        at(67, qt_round, 2, 1536)
        # O rounds qh=0: ctxn[:, 0:1024] complete once block 3's deferred
        # normalize lands (~gs 66-68); 12 rounds at gs 70..81.
        gs_o = 70
        for n0 in (0, 512):
            for of in range(NK):
                at(gs_o, o_round, of, n0)
                gs_o += 2
        o_tail_extra = []

        # ---------------- attention ----------------
        ZJ = [(0, 0), (1, 0), (0, 512), (1, 512)]  # (h2, ni_off) per zps row 32j
        BLOCKS = [(0, 0), (0, 1), (1, 0), (2, 0), (1, 1), (2, 1)]

        with tc.tile_pool(name="attn", bufs=4) as attn, \
             tc.tile_pool(name="z97", bufs=1) as z97p, \
             tc.tile_pool(name="z2", bufs=1) as z2p, \
             tc.tile_pool(name="rzbc", bufs=1) as rzbcp, \
             tc.tile_pool(name="rzd", bufs=2, space="DRAM") as rzdp, \
             tc.tile_pool(name="psS", bufs=2, space="PSUM") as psS, \
             tc.tile_pool(name="psC", bufs=1, space="PSUM") as psC, \
             tc.tile_pool(name="psZ", bufs=1, space="PSUM") as psZ, \
             tc.tile_pool(name="psK", bufs=1, space="PSUM") as psK:

            # dummy exp so the ACT table loads during the prologue DMA wait
            nc.scalar.activation(out=warm, in_=bias_sb[0:1, 0:2], func=EXP)

            # PE warm-up: HAM clock-gate release before real work
            scratch = attn.tile([128, 1024], dt.bfloat16, tag="p", name="warmup_src")
            nc.vector.memset(scratch, 0.0)
            for i in range(12):
                ps = psS.tile([128, 1024], dt.float32, tag="sps", name="sps")
                nc.tensor.matmul(ps[:, 0:512], scratch[:, 0:128],
                                 scratch[:, 0:512], start=True, stop=True)

            # prologue compute (gated only on the critical DMA set). The
            # xt-gated rounds (kt, v) run during the ~6us-longer xf DMA
            # wait; rotating the psum bank across psK/psC/psZ lets each
            # round's matmuls overlap the previous round's DVE drain.
            run_full(kt_round(psK, 0, 0))
            pv0 = psC.tile([128, 1024], dt.float32, tag="ctxp", name="pv0")
            c0 = 0 * 128
            for k in range(NK):
                nc.tensor.matmul(pv0[:, 0:VW], xt_sb[k][:, 0:128], wv_sb[k],
                                 start=(k == 0), stop=(k == NK - 1))
            nc.vector.tensor_copy(out=v_sb[0], in_=pv0[:, 0:VW])
            pv1 = psZ.tile([128, 512], dt.float32, tag="zps", name="pv1")
            for k in range(NK):
                nc.tensor.matmul(pv1[:, 0:VW], xt_sb[k][:, 128:256], wv_sb[k],
                                 start=(k == 0), stop=(k == NK - 1))
            nc.vector.tensor_copy(out=v_sb[1], in_=pv1[:, 0:VW])
            run_full(kt_round(psK, 0, 512))       # needed gs 4
            run_full(qt_round(psK, 0, 0))
            run_full(qt_round(psK, 0, 512))
            emit_bulk_dmas()

            def emit_ctx(hp, qh, st, pp, ctxp, zps):
                # ctx pairs are emitted adjacently per ni so the col-packed
                # (0,0)/(0,64) matmuls run concurrently on the PE array;
                # the Z quad (col positions 0/32/64/96) likewise.
                for ni in range(2):
                    for h2 in (0, 1):
                        nc.tensor.matmul(
                            ctxp[64 * h2:64 * h2 + 64, 512 * ni:512 * ni + 512],
                            v_sb[st][:, (2 * hp + h2) * HD:(2 * hp + h2 + 1) * HD],
                            pp[ni][:, 512 * h2:512 * h2 + 512],
                            start=(st == 0), stop=(st == NST - 1),
                            tile_position=(0, 64 * h2))
                for j, (h2, noff) in enumerate(ZJ):
                    ni = noff // 512
                    nc.tensor.matmul(
                        zps[32 * j:32 * j + 1, 0:512],
                        ones_col,
                        pp[ni][:, 512 * h2:512 * h2 + 512],
                        start=(st == 0), stop=(st == NST - 1),
                        tile_position=(0, 32 * j))

            def drain_psum(hp, qh, ctxp, zps, last=False):
                # Free psZ (DVE reciprocal direct from PSUM) and psC (ACT
                # copy) quickly; the broadcast DMA chain then runs while the
                # deferred normalize waits on GpSimd.
                q0 = qh * 1024
                rz97 = z97p.tile([97, 512], dt.float32, tag="z97", name="z97")
                nc.vector.reciprocal_approx_fast(out=rz97, in_=zps[0:97, 0:512])
                cslice = ctxn[hp][:, q0:q0 + 1024]
                # ACT copy: psC released without queueing behind the DVE's
                # mask-multiply backlog
                nc.scalar.copy(out=cslice, in_=ctxp)
                rz97h = z97p.tile([97, 512], dt.bfloat16, tag="z97h", name="z97h")
                nc.vector.tensor_copy(out=rz97h, in_=rz97)
                if last:
                    # final block: 1/Z gathered onto ONE partition; the tail
                    # broadcasts it via a K=1 PE matmul (no DRAM bounce)
                    rzf = z2p.tile([1, 2048], dt.bfloat16, tag="rzf", name="rzf")
                    for j, (h2, noff) in enumerate(ZJ):
                        eng = nc.sync if j % 2 == 0 else nc.scalar
                        eng.dma_start(out=rzf[0:1, h2 * 1024 + noff:h2 * 1024 + noff + 512],
                                      in_=rz97h[32 * j:32 * j + 1, :])
                    return cslice, rzf
                rz2h = z2p.tile([2, 1024], dt.bfloat16, tag="rz2h", name="rz2h")
                for j, (h2, noff) in enumerate(ZJ):
                    nc.sync.dma_start(out=rz2h[h2:h2 + 1, noff:noff + 512],
                                      in_=rz97h[32 * j:32 * j + 1, :])
                rzd = rzdp.tile([2, 1024], dt.bfloat16, tag="rzd", name="rzd")
                nc.sync.dma_start(out=rzd, in_=rz2h)
                bc = rzbcp.tile([128, 1024], dt.bfloat16, tag="rzbc", name="rzbc")
                srcap = rzd[0:2, :]
                bcast = bass.AP(tensor=srcap.tensor, offset=srcap.offset,
                                ap=[srcap.ap[0], [0, HD], srcap.ap[1]])
                nc.sync.dma_start(out=bc, in_=bcast)
                return cslice, bc

            pending = None
            pending_norm = []
            ctxp_cur = None
            zps_cur = None
            round_q = []

            def pump(n):
                done = 0
                while round_q and done < n:
                    try:
                        next(round_q[0])
                        done += 1
                    except StopIteration:
                        round_q.pop(0)

            for bi, (hp, qh) in enumerate(BLOCKS):
                q0 = qh * 1024
                for st in range(NST):
                    gs = bi * NST + st
                    c0 = st * 128
                    if st == 0:
                        ctxp_cur = psC.tile([128, 1024], dt.float32,
                                            tag="ctxp", name="ctxp")
                        zps_cur = psZ.tile([128, 512], dt.float32,
                                           tag="zps", name="zps")
                    # S pairs: both ni emitted back-to-back so each
                    # row-packed (0,0)/(64,0) pair runs concurrently
                    sps_t = []
                    for ni in range(2):
                        n0 = q0 + 512 * ni
                        sps = psS.tile([128, 1024], dt.float32, tag="sps", name="sps")
                        for h2 in (0, 1):
                            nc.tensor.matmul(
                                sps[:, 512 * h2:512 * h2 + 512],
                                kt_sb[hp][HD * h2:HD * h2 + HD, c0:c0 + 128],
                                qt_sb[hp][HD * h2:HD * h2 + HD, n0:n0 + 512],
                                start=True, stop=True,
                                tile_position=(64 * h2, 0))
                        sps_t.append(sps)
                    pp = []
                    for ni in range(2):
                        n0 = q0 + 512 * ni
                        p = attn.tile([128, 1024], dt.bfloat16, tag="p", name="p")
                        nc.scalar.activation(out=p, in_=sps_t[ni], func=EXP)
                        m = mask_sb[st][:, n0:n0 + 512]
                        mrep = bass.AP(tensor=m.tensor, offset=m.offset,
                                       ap=[m.ap[0], [0, 2], m.ap[1]])
                        nc.vector.tensor_mul(out=p, in0=p, in1=mrep)
                        pp.append(p)
                    # xf second-half DMAs ride the ACT queue (scalar ring),
                    # self-throttled behind the early exps
                    if gs < NK:
                        nc.scalar.dma_start(
                            out=xf_sb[gs][:, 1024:QROWS],
                            in_=xfT[gs * 128:(gs + 1) * 128, 1024:QROWS])
                    # deferred 1/Z normalizes on GpSimd (waits there without
                    # head-of-line blocking the DVE)
                    while pending_norm and pending_norm[0][0] <= gs:
                        _, cs, bcx = pending_norm.pop(0)
                        nc.gpsimd.tensor_mul(out=cs, in0=cs, in1=bcx)
                    for entry in sched.get(gs, ()):
                        round_q.append(entry[0](psK, *entry[1:]))
                    # two pump points per step: one half-lump before the ctx
                    # emission, one after — except where block 0's schedule
                    # needs 2-3 rounds in one step
                    np_ = 3 if gs in (0, 1, 2, 13, 14, 15) else 1
                    pump(np_)
                    if pending is not None:
                        php, pqh, pst, ppp, pctxp, pzps = pending
                        emit_ctx(php, pqh, pst, ppp, pctxp, pzps)
                    pump(np_)
                    pending = (hp, qh, st, pp, ctxp_cur, zps_cur)
                    if st == NST - 1:
                        emit_ctx(hp, qh, st, pp, ctxp_cur, zps_cur)
                        cs, bcx = drain_psum(hp, qh, ctxp_cur, zps_cur,
                                             last=(bi == len(BLOCKS) - 1))
                        pending_norm.append((gs + 1, cs, bcx))
                        pending = None
            pump(1000)  # flush any unfinished rounds (should be none)
            assert len(pending_norm) == 1
            _, cs6, rzf6 = pending_norm.pop()

        # tail: output projection for query half 1 with 4 PSUM banks.
        # Software-pipelined: each o_round's hp0/hp1 partial matmuls (which
        # do not depend on the final block's normalize) run ahead — the
        # first two even before the 1/Z broadcast — so the PE never idles
        # on the normalize chain or on a drain.
        with tc.tile_pool(name="psO", bufs=4, space="PSUM") as psO:
            rlist = o_tail_extra + [(of, n0) for n0 in (1024, 1536)
                                    for of in range(NK)]
            gens = []
            for i, (of, n0) in enumerate(rlist):
                gens.append(o_round(psO, of, n0,
                                    eng=(nc.sync if i % 2 == 0 else nc.gpsimd),
                                    drain_scalar=(i % 2 == 1)))
            next(gens[0])
            next(gens[1])
            # finish the last block's 1/Z normalize via K=1 PE broadcast
            for qc in (0, 512):
                bcp = psO.tile([128, 512], dt.float32, tag="psK", name="bcp")
                for h2 in range(2):
                    nc.tensor.matmul(
                        bcp[64 * h2:64 * h2 + 64, :], ones_row[0:1, 0:64],
                        rzf6[0:1, h2 * 1024 + qc:h2 * 1024 + qc + 512],
                        start=True, stop=True, tile_position=(0, 64 * h2))
                nc.vector.tensor_mul(out=cs6[:, qc:qc + 512],
                                     in0=cs6[:, qc:qc + 512], in1=bcp)
            for i in range(len(gens)):
                run_full(gens[i])
                if i + 2 < len(gens):
                    next(gens[i + 2])

        outsb_cm.__exit__(None, None, None)
        projin_cm.__exit__(None, None, None)
        persist_cm.__exit__(None, None, None)

    nc.compile()
    return nc


def _get_nc():
    global _CACHED_NC
    if _CACHED_NC is None:
        _CACHED_NC = _build_nc()
    return _CACHED_NC


def _prep_inputs(from_tensor, to_tensor, attention_mask,
                 Wq, bq, Wk, bk, Wv, bv, Wo, bo):
    f32 = np.float32
    from_tensor = np.asarray(from_tensor, f32)
    to_tensor = np.asarray(to_tensor, f32)
    attention_mask = np.asarray(attention_mask)

    Wq, bq = np.asarray(Wq, f32), np.asarray(bq, f32)
    Wk = np.asarray(Wk, f32)
    Wv, bv = np.asarray(Wv, f32), np.asarray(bv, f32)
    Wo, bo = np.asarray(Wo, f32), np.asarray(bo, f32)
    wq_s = (Wq * SCALE).astype(BF16)
    wk_h = Wk.astype(BF16)
    wv_h = Wv.astype(BF16)

    xfT_all = [np.ascontiguousarray(from_tensor[b].T).astype(BF16) for b in range(B)]
    xtT_all = [np.ascontiguousarray(to_tensor[b].T).astype(BF16) for b in range(B)]
    maskT_all = [np.ascontiguousarray(attention_mask[b].T).astype(BF16)
                 for b in range(B)]

    per_g = []
    for g in range(2):
        i0 = g * VW
        wo_g = Wo[i0:i0 + VW, :]
        # bias folds: bk dropped (constant along softmax axis); bv folded into
        # the output bias (softmax weights sum to 1); bo added by g=0 only
        bo_eff = bv[i0:i0 + VW] @ wo_g + (bo if g == 0 else 0.0)
        biases = np.zeros((128, HP + NK), f32)
        biases[:, 0:HP] = (bq[i0:i0 + VW] * SCALE).reshape(HP, 128).T
        biases[:, HP:] = bo_eff.reshape(NK, 128).T
        per_g.append({
            "wqkv": np.ascontiguousarray(np.concatenate(
                [wq_s[:, i0:i0 + VW], wk_h[:, i0:i0 + VW],
                 wv_h[:, i0:i0 + VW]], axis=1)),
            "wo": wo_g.astype(BF16),
            "biases": biases,
        })

    in_maps = []
    for c in range(NCORES):
        b, g = c // 2, c % 2
        m = {"xfT": xfT_all[b], "xtT": xtT_all[b], "maskT": maskT_all[b]}
        m.update(per_g[g])
        in_maps.append(m)
    return in_maps


def _assemble(results):
    out = np.empty((B, SF, DIM), np.float32)
    for b in range(B):
        p0 = np.asarray(results[2 * b]["out"], np.float32)
        p1 = np.asarray(results[2 * b + 1]["out"], np.float32)
        out[b] = (p0 + p1).T
    return out


def _run(in_maps, trace=False):
    from concourse.bass_utils import run_bass_kernel_spmd
    nc = _get_nc()
    return run_bass_kernel_spmd(nc, in_maps, core_ids=list(range(NCORES)),
                                trace=trace)


def kernel(**inputs):
    in_maps = _prep_inputs(**inputs)
    res = _run(in_maps, trace=False)
    return _assemble(res.results)


def kernel_profiled(**inputs):
    """Returns (output, exec_time_ns, trace_path)."""
    in_maps = _prep_inputs(**inputs)
    res = _run(in_maps, trace=True)
    trace_path = None
    if res.instructions_and_trace is not None:
        trace_path = res.instructions_and_trace[1]
    return _assemble(res.results), res.exec_time_ns, trace_path
